# revision 78
# baseline (speedup 1.0000x reference)
"""Trainium2 Bass kernel for nn_Bottleneck_CSA_ConvBlock.

Computation (per image, C=64, H=W=160):
    y  = silu(bn1(conv3x3(x, w1)))
    fq = conv3x3(y, wq); fk = conv3x3(y, wk); fv = conv3x3(y, wv)
    k_sum = fk.sum(ch, h); f_scores[c] = scale * sum_hw fq[c,h,w]*k_sum[w]
    scores = softmax_c(f_scores)
    out = x + relu(bn2(scores*fv + y))

Design: parity-split layout — partitions 0-63 hold a channel's EVEN padded
rows (slot u -> row 2u), partitions 64-127 the ODD rows (slot u -> row
2u+1). Output row pairs (r, r+1) map to PSUM halves, so every conv matmul
uses the full 128x128 PE array (K=128 contraction over channels x row
parity, M=128 over channels x out-row pair); 3/4 of weight cells are live.
conv1 runs in bf16 (6 matmuls per 3-pair group). conv_v runs in fp8e4 with
DoubleRow perf mode (K=256: 2 k-tiles = 2 row-slots per partition), halving
its PE time; per-out-channel fp8 weight scales are folded into the softmax
score vector. fq/fk never materialize: f_scores reduces to a 3-row virtual
conv over column sums (CmL/C/CmF) computed via cross-parity identity-stack
matmuls. Residual uses bf16 x (kept resident in SBUF from a single
host-padded parity DMA); fp32 accumulation in PSUM throughout.

Sharding: pure data parallelism, 2 images per core across 8 cores.
"""

import numpy as np

C = 64
H = W = 160
IMG = H * W            # 25600
WP2 = 176              # padded row stride (162 used cols + slack, 16B aligned)
NSLOT = 81             # row slots per parity half
SLOT_LEN = NSLOT * WP2  # 14256
BN_EPS = 1e-5
FP8_CONVV = True

_CACHED = {}


def _groups():
    """(k0, np): pair indices k0..k0+np-1, pair k -> out rows (2k+1, 2k+2)."""
    gs = []
    k0 = 0
    while k0 < 80:
        np_ = min(3, 80 - k0)
        gs.append((k0, np_))
        k0 += np_
    return gs


def _build_nc(loop_n=None, silu=True, unroll=1,
              phases=("dma_in", "conv1", "scores", "pass2")):
    import concourse.bass as bass
    import concourse.tile as tile
    from concourse import bacc, mybir
    from concourse.masks import make_identity

    dt = mybir.dt
    AF = mybir.ActivationFunctionType
    AX = mybir.AxisListType
    PM = mybir.MatmulPerfMode
    f32 = dt.float32
    bf16 = dt.bfloat16
    fp8 = dt.float8e4
    ACT1 = AF.Silu if silu else AF.Identity

    nc = bacc.Bacc("TRN2", target_bir_lowering=False, debug=False, num_devices=8)

    xpar_d = nc.dram_tensor("xpar", [128, 2 * SLOT_LEN], bf16, kind="ExternalInput")
    w1_d = nc.dram_tensor("w1t", [128, 6, 128], bf16, kind="ExternalInput")
    if FP8_CONVV:
        wv_d = nc.dram_tensor("wvt", [128, 3, 2, 128], fp8, kind="ExternalInput")
    else:
        wv_d = nc.dram_tensor("wvt", [128, 6, 128], bf16, kind="ExternalInput")
    wq_d = nc.dram_tensor("wqt", [64, 9, 65], bf16, kind="ExternalInput")
    ii_d = nc.dram_tensor("iit", [128, 64], f32, kind="ExternalInput")
    bn1s_d = nc.dram_tensor("bn1s", [128, 1], f32, kind="ExternalInput")
    bn1b_d = nc.dram_tensor("bn1b", [128, 1], f32, kind="ExternalInput")
    bn2s_d = nc.dram_tensor("bn2s", [128, 1], f32, kind="ExternalInput")
    bn2b_d = nc.dram_tensor("bn2b", [128, 1], f32, kind="ExternalInput")
    invs_d = nc.dram_tensor("invst", [1, 64], f32, kind="ExternalInput")
    # out layout: row = 64*img + ch; cols [0:12800] odd image rows (1,3,..159),
    # cols [12800:25600] even image rows (0,2,..158). Host de-interleaves.
    out_d = nc.dram_tensor("out", [128, IMG], bf16, kind="ExternalOutput")

    groups = _groups()

    with tile.TileContext(nc) as tc:
        ctx_lp = nc.allow_low_precision("bf16/fp8 matmul path; fp32 PSUM accumulation")
        ctx_lp.__enter__()
        with (
            tc.tile_pool(name="const", bufs=1) as const,
            tc.tile_pool(name="persist", bufs=1) as persist,
            tc.tile_pool(name="small", bufs=2) as small,
            tc.tile_pool(name="epi", bufs=5) as epi,
            tc.tile_pool(name="psmain", bufs=6, space="PSUM") as psmain,
            tc.tile_pool(name="pssc", bufs=1, space="PSUM") as pssc,
        ):
            w1_sb = const.tile([128, 6, 128], bf16)
            nc.sync.dma_start(out=w1_sb[:], in_=w1_d.ap())
            if FP8_CONVV:
                wv_sb = const.tile([128, 3, 2, 128], fp8)
            else:
                wv_sb = const.tile([128, 6, 128], bf16)
            nc.sync.dma_start(out=wv_sb[:], in_=wv_d.ap())
            wq_sb = const.tile([64, 9, 65], bf16)
            nc.sync.dma_start(out=wq_sb[:], in_=wq_d.ap())
            ii_sb = const.tile([128, 64], f32)
            nc.sync.dma_start(out=ii_sb[:], in_=ii_d.ap())
            bn1s = const.tile([128, 1], f32)
            nc.sync.dma_start(out=bn1s[:], in_=bn1s_d.ap())
            bn1b = const.tile([128, 1], f32)
            nc.sync.dma_start(out=bn1b[:], in_=bn1b_d.ap())
            bn2s = const.tile([128, 1], f32)
            nc.sync.dma_start(out=bn2s[:], in_=bn2s_d.ap())
            bn2b = const.tile([128, 1], f32)
            nc.sync.dma_start(out=bn2b[:], in_=bn2b_d.ap())
            invs_sb = const.tile([1, 64], f32)
            nc.sync.dma_start(out=invs_sb[:], in_=invs_d.ap())
            ident = const.tile([128, 128], f32)
            make_identity(nc, ident[:])
            ones_sb = const.tile([128, 64], bf16)
            nc.vector.memset(ones_sb[:], 1.0)

            FIMG = 80 * W          # 12800: flat per-image length per parity
            x_t = persist.tile([128, 2 * SLOT_LEN], bf16)
            # flat group-major y: [p, img*FIMG + v*160 + col]; lower v = image
            # odd row 2v+1, upper v = image even row 2v. All elementwise ops
            # on this layout are full 128-partition width.
            # DVE casts yflat->y8f (fp8), hardware DGE scatters into the
            # padded slot layout. (A gpsimd casting-DMA variant models ~1.5us
            # faster but relies on software-DGE speed on real HW.)
            gp_scatter = False
            yflat = persist.tile([128, 2 * FIMG], bf16)
            if FP8_CONVV:
                if not gp_scatter:
                    y8f = persist.tile([128, 2 * FIMG], fp8)
                y8_t = persist.tile([128, 2 * SLOT_LEN], fp8)
                # y8 pads: col 0/161 of every slot; even slot 0; odd slot 80
                y4 = y8_t[:].rearrange("p (i u c) -> p i u c", i=2, c=WP2)
                nc.vector.memset(y4[:, :, :, 0:1], 0.0)
                nc.vector.memset(y4[:, :, :, 161:162], 0.0)
                nc.vector.memset(y4[0:64, :, 0, 0:162], 0.0)
                nc.vector.memset(y4[64:128, :, 80, 0:162], 0.0)
            else:
                y8f = None
                y8_t = persist.tile([128, 2 * SLOT_LEN], bf16)
                y4 = y8_t[:].rearrange("p (i u c) -> p i u c", i=2, c=WP2)
                nc.vector.memset(y4[:, :, :, 0:1], 0.0)
                nc.vector.memset(y4[:, :, :, 161:162], 0.0)
                nc.vector.memset(y4[0:64, :, 0, 0:162], 0.0)
                nc.vector.memset(y4[64:128, :, 80, 0:162], 0.0)
            scores_t = persist.tile([128, 2], f32)

            def conv_group(img, k0, np_, w_sb, is_v):
                base = img * SLOT_LEN
                src = y8_t if is_v else x_t
                ps = psmain.tile([128, 3 * W], f32, tag="ps")
                N = np_ * W
                if is_v and FP8_CONVV:
                    for p in range(np_):
                        off = base + (k0 + p) * WP2
                        rhs = y8_t[:, off:off + 2 * WP2].rearrange(
                            "p (t c) -> p t c", t=2)
                        for dx in range(3):
                            nc.tensor.matmul(
                                ps[:, p * W:(p + 1) * W],
                                w_sb[:, dx, :, :],
                                rhs[:, :, dx:dx + 160],
                                start=(dx == 0), stop=(dx == 2),
                                perf_mode=PM.DoubleRow,
                            )
                else:
                    for dx in range(3):
                        for inst in range(2):
                            off = base + (k0 + inst) * WP2
                            rhs = src[:, off:off + np_ * WP2].rearrange(
                                "p (r c) -> p r c", c=WP2)[:, :, dx:dx + 160]
                            nc.tensor.matmul(
                                ps[:, :N],
                                w_sb[:, 2 * dx + inst, :],
                                rhs,
                                start=(dx == 0 and inst == 0),
                                stop=(dx == 2 and inst == 1),
                            )
                return ps

            def slot_ap(t, img, u0, np_, half):
                """[64, np_, 160] AP at interior cols of slots u0..u0+np_-1."""
                base = img * SLOT_LEN + u0 * WP2
                sl = t[64 * half:64 * half + 64, base:base + np_ * WP2]
                return sl.rearrange("p (r c) -> p r c", c=WP2)[:, :, 1:161]

            def y_scatter(img, b0, b1):
                """Scatter (and cast) yflat pairs b0..b1-1 into the padded
                slot layout of y8_t via gpsimd software-DGE DMA."""
                npair = b1 - b0
                fl = img * FIMG + b0 * W
                if FP8_CONVV and gp_scatter:
                    eng, src = nc.gpsimd, yflat
                elif FP8_CONVV:
                    eng, src = nc.sync, y8f
                else:
                    eng, src = nc.sync, yflat
                eng.dma_start(
                    out=slot_ap(y8_t, img, b0 + 1, npair, 0),
                    in_=src[0:64, fl:fl + npair * W].rearrange(
                        "p (r c) -> p r c", c=W))
                eng.dma_start(
                    out=slot_ap(y8_t, img, b0, npair, 1),
                    in_=src[64:128, fl:fl + npair * W].rearrange(
                        "p (r c) -> p r c", c=W))

            def conv1_group(img, k0, np_):
                ps = conv_group(img, k0, np_, w1_sb, False)
                N = np_ * W
                fl = img * FIMG + k0 * W
                nc.scalar.activation(
                    out=yflat[:, fl:fl + N],
                    in_=ps[:, :N],
                    func=ACT1, bias=bn1b[:], scale=bn1s[:])
                if FP8_CONVV and not gp_scatter:
                    nc.vector.tensor_copy(y8f[:, fl:fl + N], yflat[:, fl:fl + N])

            def conv1_img(img):
                done = 0
                for gi, (k0, np_) in enumerate(groups):
                    conv1_group(img, k0, np_)
                    if gi % 3 == 2 or gi == len(groups) - 1:
                        end = k0 + np_
                        y_scatter(img, done, end)
                        done = end

            def scores_stages(img):
                """Generator: emit the scores chain in stages so the caller
                can sprinkle PE-heavy groups between dependent steps."""
                fb = img * FIMG
                parts = small.tile([128, 3 * 162], f32, tag="parts")
                csc = small.tile([128, 160], f32, tag="csc")
                nc.gpsimd.memset(parts[:], 0.0)

                def colr(v0, v1):
                    return yflat[:, fb + v0 * W:fb + v1 * W].rearrange(
                        "p (r c) -> p r c", c=W).rearrange(
                        "p r c -> p c r")

                # col sums over rows: lower lanes sum odd rows, upper even
                nc.vector.reduce_sum(parts[:, 1:161], colr(0, 40), axis=AX.X)
                yield
                nc.vector.reduce_sum(csc[:], colr(40, 80), axis=AX.X)
                yield
                nc.gpsimd.tensor_add(parts[:, 1:161], parts[:, 1:161], csc[:])
                yield
                # CmL = C - row160 (image row 159 = lower v79);
                # CmF = C - row1 (image row 0 = upper v0)
                nc.vector.tensor_sub(parts[0:64, 163:323], parts[0:64, 1:161],
                                     yflat[0:64, fb + 79 * W:fb + 80 * W])
                nc.gpsimd.tensor_copy(parts[64:128, 163:323], parts[64:128, 1:161])
                nc.vector.tensor_copy(parts[0:64, 325:485], parts[0:64, 1:161])
                nc.gpsimd.tensor_sub(parts[64:128, 325:485], parts[64:128, 1:161],
                                     yflat[64:128, fb:fb + W])
                yield
                qsp_t = pssc.tile([128, 3 * 162], f32, tag="sc")
                qsp = qsp_t[0:64, :]
                nc.tensor.matmul(qsp, ii_sb[:], parts[:, :],
                                 start=True, stop=True)
                yield
                q_src = small.tile([64, 3 * 162], bf16, tag="q_src")
                nc.vector.tensor_copy(q_src[:], qsp)
                yield
                # virtual 3-row conv: dy0 -> CmL(162), dy1 -> C(0), dy2 -> CmF(324)
                dy_base = {0: 162, 1: 0, 2: 324}
                scrap = pssc.tile([128, 3 * 162], f32, tag="sc")
                qp = scrap[0:65, 0:160]
                for k9 in range(9):
                    dy, dx = divmod(k9, 3)
                    b = dy_base[dy]
                    nc.tensor.matmul(qp, wq_sb[:, k9, :],
                                     q_src[:, b + dx:b + dx + 160],
                                     start=(k9 == 0), stop=(k9 == 8))
                yield
                q_s = small.tile([65, 160], bf16, tag="q_s")
                nc.vector.tensor_copy(q_s[:], qp)
                yield
                bcp = scrap[0:64, 160:320]
                nc.tensor.matmul(bcp, ones_sb[64:65, :], q_s[64:65, :],
                                 start=True, stop=True, tile_position=(64, 0))
                yield
                t_s = small.tile([64, 160], f32, tag="t_s")
                nc.vector.tensor_mul(t_s[:], q_s[0:64, :], bcp)
                fs = small.tile([64, 1], f32, tag="fs")
                nc.vector.reduce_sum(fs[:], t_s[:], axis=AX.X)
                yield
                trp = scrap[0:1, 320:384]
                nc.tensor.transpose(trp, fs[:], ident[0:64, 0:64])
                yield
                frow = small.tile([1, 64], f32, tag="frow")
                nc.vector.tensor_copy(frow[:], trp)
                mx = small.tile([1, 1], f32, tag="mx")
                nc.vector.reduce_max(mx[:], frow[:], axis=AX.X, negate=True)
                srow = small.tile([1, 64], f32, tag="srow")
                nc.scalar.activation(out=srow[:], in_=frow[:], func=AF.Exp,
                                     bias=mx[:], scale=1.0)
                sm = small.tile([1, 1], f32, tag="sm")
                nc.vector.reduce_sum(sm[:], srow[:], axis=AX.X)
                rs = small.tile([1, 1], f32, tag="rs")
                nc.vector.reciprocal(rs[:], sm[:])
                nc.vector.tensor_scalar_mul(srow[:], srow[:], rs[:])
                nc.vector.tensor_mul(srow[:], srow[:], invs_sb[:])
                srowd = small.tile([1, 128], f32, tag="srowd")
                nc.vector.tensor_copy(srowd[0:1, 0:64], srow[:])
                nc.vector.tensor_copy(srowd[0:1, 64:128], srow[:])
                yield
                psc = scrap[0:128, 384:385]
                nc.tensor.transpose(psc, srowd[:], ident[0:1, 0:1])
                yield
                nc.vector.tensor_copy(scores_t[:, img:img + 1], psc)

            def scores_img(img):
                for _ in scores_stages(img):
                    pass

            otbatch = [None, 0]  # current otflat tile, pair base

            def pass2_group(img, k0, np_):
                ps = conv_group(img, k0, np_, wv_sb, True)
                N = np_ * W
                fl = img * FIMG + k0 * W
                u = epi.tile([128, 3 * W], bf16, tag="u")
                if (k0 // 3) % 2 == 0:
                    nc.scalar.mul(u[:, :N], ps[:, :N], scores_t[:, img:img + 1])
                else:
                    nc.vector.tensor_scalar_mul(u[:, :N], ps[:, :N],
                                                scores_t[:, img:img + 1])
                u2 = epi.tile([128, 3 * W], bf16, tag="u2")
                nc.vector.tensor_add(u2[:, :N], u[:, :N], yflat[:, fl:fl + N])
                rt = epi.tile([128, 3 * W], bf16, tag="rt")
                nc.scalar.activation(out=rt[:, :N], in_=u2[:, :N],
                                     func=AF.Relu, bias=bn2b[:], scale=bn2s[:])
                if otbatch[0] is None:
                    otbatch[0] = epi.tile([128, 9 * W], bf16, tag="ot", name="otb")
                    otbatch[1] = k0
                ot = otbatch[0]
                lo = (k0 - otbatch[1]) * W
                otr = ot[:, lo:lo + N].rearrange("p (r c) -> p r c", c=W)
                rtr = rt[:, :N].rearrange("p (r c) -> p r c", c=W)
                nc.gpsimd.tensor_add(otr[0:64], rtr[0:64],
                                     slot_ap(x_t, img, k0 + 1, np_, 0))
                nc.vector.tensor_add(otr[64:128], rtr[64:128],
                                     slot_ap(x_t, img, k0, np_, 1))
                end = k0 + np_
                if end - otbatch[1] >= 9 or end >= 80:
                    b0 = otbatch[1]
                    NB = (end - b0) * W
                    oap = out_d.ap()
                    nc.sync.dma_start(
                        out=oap[64 * img:64 * img + 64, b0 * W:b0 * W + NB],
                        in_=ot[0:64, :NB])
                    nc.sync.dma_start(
                        out=oap[64 * img:64 * img + 64,
                                IMG // 2 + b0 * W:IMG // 2 + b0 * W + NB],
                        in_=ot[64:128, :NB])
                    otbatch[0] = None

            def pass2_img(img):
                for (k0, np_) in groups:
                    pass2_group(img, k0, np_)

            def body():
                xap = xpar_d.ap()
                if "dma_in" in phases:
                    for img in range(2):
                        b0 = img * SLOT_LEN
                        for c0, c1 in ((0, 7), (7, 21), (21, 42), (42, 63), (63, NSLOT)):
                            nc.sync.dma_start(
                                out=x_t[:, b0 + c0 * WP2:b0 + c1 * WP2],
                                in_=xap[:, b0 + c0 * WP2:b0 + c1 * WP2])
                full = all(p in phases for p in ("conv1", "scores", "pass2"))
                if full:
                    # pipeline: conv1(0); conv1(1) hiding scores(0) then
                    # overlapping pass2(0); pass2(0) tail hiding scores(1);
                    # pass2(1).
                    SENT = object()
                    conv1_img(0)
                    sc0 = scores_stages(0)
                    done0 = False
                    p2i = 0
                    done = 0
                    for gi, (k0, np_) in enumerate(groups):
                        conv1_group(1, k0, np_)
                        if gi % 3 == 2 or gi == len(groups) - 1:
                            y_scatter(1, done, k0 + np_)
                            done = k0 + np_
                        if not done0:
                            done0 = next(sc0, SENT) is SENT
                            if not done0:
                                done0 = next(sc0, SENT) is SENT
                        else:
                            pass2_group(0, *groups[p2i])
                            p2i += 1
                    for _ in sc0:
                        pass
                    sc1 = scores_stages(1)
                    done1 = False
                    while p2i < len(groups):
                        pass2_group(0, *groups[p2i])
                        p2i += 1
                        if not done1:
                            done1 = next(sc1, SENT) is SENT
                            if not done1:
                                done1 = next(sc1, SENT) is SENT
                    for _ in sc1:
                        pass
                    pass2_img(1)
                else:
                    for img in range(2):
                        if "conv1" in phases:
                            conv1_img(img)
                        if "scores" in phases:
                            scores_img(img)
                        if "pass2" in phases:
                            pass2_img(img)
                if "pass2" not in phases:
                    # keep an output write so the NEFF isn't degenerate
                    nc.sync.dma_start(out=out_d.ap()[:, 0:64], in_=ones_sb[:])

            if loop_n is not None:
                with tc.For_i(0, loop_n, 1):
                    for _ in range(unroll):
                        body()
            else:
                body()
        ctx_lp.__exit__(None, None, None)
    nc.compile()
    return nc


def _build_null():
    import concourse.tile as tile
    from concourse import bacc, mybir

    nc = bacc.Bacc("TRN2", target_bir_lowering=False, debug=False, num_devices=8)
    out_d = nc.dram_tensor("out", [128, 4], mybir.dt.float32, kind="ExternalOutput")
    with tile.TileContext(nc) as tc:
        with tc.tile_pool(name="p", bufs=1) as p:
            t = p.tile([128, 4], mybir.dt.float32)
            nc.vector.memset(t[:], 0.0)
            nc.sync.dma_start(out=out_d.ap(), in_=t[:])
    nc.compile()
    return nc


def _get_nc():
    if "nc" not in _CACHED:
        _CACHED["nc"] = _build_nc()
    return _CACHED["nc"]


def _prep_weights(w_cv1, wq, wk, wv, g1, b1, m1, v1, g2, b2, m2, v2):
    import ml_dtypes
    bf = ml_dtypes.bfloat16
    f8 = ml_dtypes.float8_e4m3fn

    def pair_blocks(w):
        """[O,I,3,3] -> [6, 128, 128]: (I1,I2) x dx; lhsT[p,m] block layout."""
        T = w.transpose(1, 0, 2, 3)  # [i, o, dy, dx]
        out = np.zeros((6, 128, 128), np.float32)
        for dx in range(3):
            I1 = out[2 * dx]
            I2 = out[2 * dx + 1]
            I1[0:64, 64:128] = T[:, :, 0, dx]
            I1[64:128, 0:64] = T[:, :, 0, dx]
            I1[64:128, 64:128] = T[:, :, 1, dx]
            I2[0:64, 0:64] = T[:, :, 1, dx]
            I2[0:64, 64:128] = T[:, :, 2, dx]
            I2[64:128, 0:64] = T[:, :, 2, dx]
        return out

    w1t = np.ascontiguousarray(
        pair_blocks(w_cv1).transpose(1, 0, 2).astype(bf))  # [128, 6, 128]

    if FP8_CONVV:
        s = 192.0 / np.abs(wv).max(axis=(1, 2, 3))          # per out-channel
        wvs = wv * s[:, None, None, None]
        T = wvs.transpose(1, 0, 2, 3)  # [i, o, dy, dx]
        wvt = np.zeros((128, 3, 2, 128), np.float32)
        for h in range(2):
            for t in range(2):
                for dx in range(3):
                    dy_o = 2 * t + h       # m>=64: out row r (odd)
                    dy_e = 2 * t + h - 1   # m<64: out row r+1 (even)
                    if 0 <= dy_o <= 2:
                        wvt[64 * h:64 * h + 64, dx, t, 64:128] = T[:, :, dy_o, dx]
                    if 0 <= dy_e <= 2:
                        wvt[64 * h:64 * h + 64, dx, t, 0:64] = T[:, :, dy_e, dx]
        wvt = np.ascontiguousarray(wvt.astype(f8))
        invs = np.ascontiguousarray((1.0 / s).reshape(1, 64).astype(np.float32))
    else:
        wvt = np.ascontiguousarray(
            pair_blocks(wv).transpose(1, 0, 2).astype(bf))
        invs = np.ones((1, 64), np.float32)

    scale = 1.0 / (float(W) ** 0.5 * float(H) * float(H))
    q = wq.transpose(1, 2, 3, 0).reshape(C, 9, C) * scale    # [j, 9, c]
    ks = wk.sum(axis=0).reshape(C, 9, 1)                     # [j, 9, 1]
    wqt = np.ascontiguousarray(np.concatenate([q, ks], axis=2).astype(bf))

    iit = np.ascontiguousarray(
        np.vstack([np.eye(64), np.eye(64)]).astype(np.float32))

    s1 = (g1 / np.sqrt(v1 + BN_EPS)).astype(np.float32)
    b1p = (b1 - m1 * s1).astype(np.float32)
    s2 = (g2 / np.sqrt(v2 + BN_EPS)).astype(np.float32)
    b2p = (b2 - m2 * s2).astype(np.float32)

    def dup(v):
        return np.ascontiguousarray(
            np.concatenate([v, v]).reshape(128, 1).astype(np.float32))

    return dict(w1t=w1t, wvt=wvt, wqt=wqt, iit=iit, invst=invs,
                bn1s=dup(s1), bn1b=dup(b1p), bn2s=dup(s2), bn2b=dup(b2p))


def _parity_pack(x2):
    """x2: [2, 64, H, W] f32 -> [128, 2*SLOT_LEN] bf16 parity-padded."""
    import ml_dtypes
    bf = ml_dtypes.bfloat16
    out = np.zeros((128, 2, NSLOT, WP2), np.float32)
    xp = np.zeros((2, 64, 162, WP2), np.float32)
    xp[:, :, 1:161, 1:161] = x2
    out[0:64, :, :, :] = xp[:, :, 0::2, :].transpose(1, 0, 2, 3)
    out[64:128, :, :, :] = xp[:, :, 1::2, :].transpose(1, 0, 2, 3)
    return np.ascontiguousarray(out.reshape(128, 2 * SLOT_LEN).astype(bf))


def _ensure_axon_devices():
    """Make sure jax can see the 8 axon-tunneled NeuronCores even if the
    calling process pinned JAX_PLATFORMS=cpu before importing us."""
    import os
    envp = os.environ.get("JAX_PLATFORMS", "")
    if envp and "axon" not in envp:
        os.environ.pop("JAX_PLATFORMS", None)
    import jax
    try:
        devs = jax.devices()
        if len(devs) >= 8 and all("cpu" not in str(d).lower() for d in devs[:8]):
            return
    except Exception:
        pass
    try:
        from jax._src import xla_bridge
        xla_bridge.backends.cache_clear()
    except Exception:
        pass
    try:
        import jax.extend.backend as jeb
        jeb.clear_backends()
    except Exception:
        pass


def kernel(x, w_cv1, g1, b1, m1, v1, wq, wk, wv, g2, b2, m2, v2):
    _ensure_axon_devices()
    from concourse.bass_utils import run_bass_kernel_spmd

    x = np.asarray(x, dtype=np.float32)
    consts = _prep_weights(
        np.asarray(w_cv1, np.float32), np.asarray(wq, np.float32),
        np.asarray(wk, np.float32), np.asarray(wv, np.float32),
        np.asarray(g1, np.float32), np.asarray(b1, np.float32),
        np.asarray(m1, np.float32), np.asarray(v1, np.float32),
        np.asarray(g2, np.float32), np.asarray(b2, np.float32),
        np.asarray(m2, np.float32), np.asarray(v2, np.float32))
    nc = _get_nc()
    in_maps = []
    for i in range(8):
        m = {"xpar": _parity_pack(x[2 * i:2 * i + 2])}
        m.update(consts)
        in_maps.append(m)
    res = run_bass_kernel_spmd(nc, in_maps, core_ids=list(range(8)))
    out = np.empty((16, C, H, W), np.float32)
    for i, r in enumerate(res.results):
        buf = r["out"].reshape(2, C, 2, H // 2, W)
        out[2 * i:2 * i + 2, :, 1::2, :] = buf[:, :, 0]
        out[2 * i:2 * i + 2, :, 0::2, :] = buf[:, :, 1]
    return out
